# revision 1
# baseline (speedup 1.0000x reference)
"""HGAT kernel: data-parallel over recipe nodes across 8 NeuronCores.

Sharding (per spec hint): recipe axis (20000) split 8 x 2500; user-side
work + attention weights + embedding tables replicated on every core.
Each core computes H_U [2048,64] (replicated), its recipe shard
H_R [2500,64], and the pred block [2048,2500]; host concatenates.
"""
import numpy as np
import jax
import jax.numpy as jnp

LEAK = 0.2
N_CORES = 8
N_RECIPE = 20000
SHARD = N_RECIPE // N_CORES  # 2500

_compiled = None


def _node_attn(hc, nb, Ws, Wn, a):
    Wc = jnp.einsum('nc,hco->nho', hc, Ws)
    Wb = jnp.einsum('nkc,hco->nhko', nb, Wn)
    s = jax.nn.leaky_relu(
        jnp.einsum('nho,ho->nh', Wc, a[:, 0])[:, :, None]
        + jnp.einsum('nhko,ho->nhk', Wb, a[:, 1]), LEAK)
    alpha = jax.nn.softmax(s, axis=-1)
    out = jax.nn.elu(jnp.einsum('nhk,nhko->nho', alpha, Wb))
    N, H, O = out.shape
    return out.reshape(N, H * O)


def _shard_fwd(uemb_sel, ur_nb, rec_cen, ru_s, rr_s, ri_s,
               user_emb, recipe_emb, ing_emb,
               Ws_u, Wn_u, a_u, Ws_ru, Wn_ru, a_ru, Ws_rr, Wn_rr, a_rr,
               Ws_ri, Wn_ri, a_ri, W_u, b_u, W_r, b_r, W_rel, b_rel, q_rel):
    # user side (replicated on every core)
    hu = _node_attn(uemb_sel, recipe_emb[ur_nb], Ws_u, Wn_u, a_u)
    H_U = hu @ W_u + b_u                                   # [B, 64]
    # recipe side, this core's shard of 2500 recipes
    hru = _node_attn(rec_cen, user_emb[ru_s], Ws_ru, Wn_ru, a_ru) @ W_r + b_r
    hrr = _node_attn(rec_cen, recipe_emb[rr_s], Ws_rr, Wn_rr, a_rr) @ W_r + b_r
    hri = _node_attn(rec_cen, ing_emb[ri_s], Ws_ri, Wn_ri, a_ri) @ W_r + b_r
    S = jnp.stack([hru, hrr, hri])                         # [3, n, 64]
    score = jnp.einsum('rno,o->rn',
                       jnp.tanh(jnp.einsum('rno,op->rnp', S, W_rel) + b_rel),
                       q_rel)
    beta = jax.nn.softmax(score, axis=0)
    H_R = jnp.einsum('rn,rno->no', beta, S)                # [n, 64]
    return H_U @ H_R.T                                     # [B, n]


def kernel(user, item_seq, ur_idx, ru_idx, rr_idx, ri_idx,
           user_emb, recipe_emb, ing_emb,
           Ws_u, Wn_u, a_u, Ws_ru, Wn_ru, a_ru, Ws_rr, Wn_rr, a_rr,
           Ws_ri, Wn_ri, a_ri, W_u, b_u, W_r, b_r, W_rel, b_rel, q_rel):
    global _compiled
    user = np.asarray(user).astype(np.int32)
    ur_idx = np.asarray(ur_idx).astype(np.int32)
    user_emb = np.asarray(user_emb, dtype=np.float32)
    recipe_emb = np.asarray(recipe_emb, dtype=np.float32)
    ing_emb = np.asarray(ing_emb, dtype=np.float32)

    # host-side index marshalling: select per-user neighbor lists, shard recipes
    uemb_sel = user_emb[user]                      # [2048, 64]
    ur_nb = ur_idx[user]                           # [2048, 32]

    def shard(a):
        return np.asarray(a).astype(np.int32).reshape(N_CORES, SHARD, -1)

    ru_s = shard(ru_idx)
    rr_s = shard(rr_idx)
    ri_s = shard(ri_idx)
    rec_cen = recipe_emb.reshape(N_CORES, SHARD, 64)

    def rep(a):
        a = np.asarray(a, dtype=np.float32)
        return np.broadcast_to(a, (N_CORES,) + a.shape)

    args = (rep(uemb_sel), np.broadcast_to(ur_nb, (N_CORES,) + ur_nb.shape),
            rec_cen, ru_s, rr_s, ri_s,
            rep(user_emb), rep(recipe_emb), rep(ing_emb),
            rep(Ws_u), rep(Wn_u), rep(a_u), rep(Ws_ru), rep(Wn_ru), rep(a_ru),
            rep(Ws_rr), rep(Wn_rr), rep(a_rr), rep(Ws_ri), rep(Wn_ri), rep(a_ri),
            rep(W_u), rep(b_u), rep(W_r), rep(b_r), rep(W_rel), rep(b_rel),
            rep(q_rel))

    if _compiled is None:
        _compiled = jax.pmap(_shard_fwd, devices=jax.devices()[:N_CORES])
    out = _compiled(*args)                          # [8, 2048, 2500]
    pred = np.concatenate(list(np.asarray(out)), axis=1)  # [2048, 20000]
    return pred.reshape(2048, 1, N_RECIPE, 1).astype(np.float32)
